# revision 35
# baseline (speedup 1.0000x reference)
import numpy as np
import ml_dtypes

from concourse import bass, tile
from concourse import bacc
from concourse import mybir
from concourse.bass_utils import run_bass_kernel_spmd
from concourse.masks import make_identity

dt = mybir.dt
AF = mybir.ActivationFunctionType

B, H, N, D = 4, 8, 2048, 64
NCORES = 8
HPC = 4          # heads per core
NT = N // 128    # 16 n-tiles of 128

_bf16 = ml_dtypes.bfloat16


def _build_nc():
    nc = bacc.Bacc("TRN2", target_bir_lowering=False)
    qk_d = nc.dram_tensor("qk", [HPC, 128, N], dt.bfloat16, kind="ExternalInput")
    v_d = nc.dram_tensor("vt", [HPC, 128, NT * D], dt.bfloat16, kind="ExternalInput")
    # xyz | W1.T | W2.T | W3.T packed: one DMA descriptor instead of four
    # (HWDGE descriptor-gen is a shared 625ns/DMA resource on the startup
    # critical path).
    wx_d = nc.dram_tensor("wx", [3, N + 3 * HPC * D], dt.bfloat16,
                          kind="ExternalInput")
    out_d = nc.dram_tensor("out", [HPC, 128, NT, D], dt.float32,
                           kind="ExternalOutput")
    W1O, W2O, W3O = N, N + HPC * D, N + 2 * HPC * D

    with tile.TileContext(nc) as tc:
        with (
            tc.tile_pool(name="const", bufs=1) as cpool,
            tc.tile_pool(name="qk", bufs=2) as qk_pool,
            tc.tile_pool(name="ab", bufs=8) as ab_pool,
            tc.tile_pool(name="vraw", bufs=2) as vraw_pool,
            tc.tile_pool(name="vp", bufs=8) as vp_pool,
            tc.tile_pool(name="expb", bufs=2 * NT) as ex_pool,
            tc.tile_pool(name="rec", bufs=4) as rec_pool,
            tc.tile_pool(name="pse", bufs=2, space="PSUM") as psum_e,
            tc.tile_pool(name="psp", bufs=2, space="PSUM") as psum_p,
            tc.tile_pool(name="pso", bufs=2, space="PSUM") as psum_o,
        ):
            # PE warmup: ~3.4us of dependency-free matmuls so the p-state ramp
            # (peak after 3us continuous busy) completes before the first real
            # matmul; otherwise the whole startup chain runs at low/mid clock.
            z128 = cpool.tile([128, 128], dt.bfloat16)
            nc.vector.memset(z128, 0.0)
            z512 = cpool.tile([128, 512], dt.bfloat16)
            nc.vector.memset(z512, 0.0)
            warm = psum_p.tile([128, 512], dt.float32, tag="pp")
            for i in range(7):
                nc.tensor.matmul(warm, z128, z512, start=(i == 0), stop=(i == 6))

            ident_bf = cpool.tile([128, 128], dt.bfloat16)
            make_identity(nc, ident_bf)
            wx_sb = cpool.tile([3, N + 3 * HPC * D], dt.bfloat16)
            nc.scalar.dma_start(out=wx_sb, in_=wx_d[:])
            xyz_sb = wx_sb[:, 0:N]
            out_p = [cpool.tile([128, NT, D], dt.float32, name=f"out_p{i}")
                     for i in range(HPC)]

            ex_tiles: dict[int, list] = {}
            vp_tiles: dict[int, list] = {}
            ab_tiles: dict[int, list] = {}
            qk_tiles: dict[int, object] = {}
            v_tiles: dict[int, object] = {}

            def emit_dmas(p):
                # qk in two halves so the first eT matmuls can start after
                # half the transfer (matters for pair 0 on the startup path)
                qk_sb = qk_pool.tile([128, N], dt.bfloat16)
                nc.sync.dma_start(out=qk_sb[:, 0:N // 2], in_=qk_d[p, :, 0:N // 2])
                nc.sync.dma_start(out=qk_sb[:, N // 2:N], in_=qk_d[p, :, N // 2:N])
                qk_tiles[p] = qk_sb
                v_sb = vraw_pool.tile([128, NT, D], dt.bfloat16)
                nc.sync.dma_start(out=v_sb, in_=v_d[p])
                v_tiles[p] = v_sb
                ab_tiles[p] = []
                vp_tiles[p] = []

            def emit_ab_chunk(p, c4):
                # AB = vstack(kT + q_pe, k_pe), bf16.  kT folded in via PE
                # identity-accumulate so the DVE copy has a single producer.
                hs = slice(p * D, (p + 1) * D)
                s = slice(c4 * 512, (c4 + 1) * 512)
                qk_sb = qk_tiles[p]
                pp = psum_p.tile([128, 512], dt.float32, tag="pp")
                nc.tensor.matmul(pp[0:64], wx_sb[:, W1O + p * D:W1O + (p + 1) * D],
                                 xyz_sb[:, s], start=True, stop=False)
                nc.tensor.matmul(pp[0:64], ident_bf[64:128, 64:128],
                                 qk_sb[64:128, s], start=False, stop=True)
                nc.tensor.matmul(pp[64:128], wx_sb[:, W3O + p * D:W3O + (p + 1) * D],
                                 xyz_sb[:, s], start=True, stop=True)
                ab = ab_pool.tile([128, 512], dt.bfloat16, tag="ab")
                nc.vector.tensor_copy(ab, pp)
                ab_tiles[p].append(ab)

            def emit_vp_chunk(p, c4):
                # vp = [v + v_peT | 1], bf16  [128, 4, 65]; v folded in via PE
                pv = psum_p.tile([128, 4, D], dt.float32, tag="pp")
                for j in range(4):
                    t = c4 * 4 + j
                    nc.tensor.matmul(pv[:, j, :], xyz_sb[:, t * 128:(t + 1) * 128],
                                     wx_sb[:, W2O + p * D:W2O + (p + 1) * D],
                                     start=True, stop=False)
                    nc.tensor.matmul(pv[:, j, :], ident_bf,
                                     v_tiles[p][:, t, :], start=False, stop=True)
                vp = vp_pool.tile([128, 4, D + 1], dt.bfloat16, tag="vp")
                nc.vector.memset(vp[:, :, D], 1.0)
                nc.vector.tensor_copy(vp[:, :, 0:D], pv)
                vp_tiles[p].append(vp)

            def emit_v(pp, nt):
                # direct-layout V pass: po[n, d] = sum_m ex[m, n] * vp[m, d];
                # col 64 (ones column of vp) = softmax denominator per row.
                po = psum_o.tile([128, D + 1], dt.float32, tag="po")
                for mt in range(NT):
                    nc.tensor.matmul(po, ex_tiles[pp][mt][:, nt * 128:(nt + 1) * 128],
                                     vp_tiles[pp][mt // 4][:, mt % 4, :],
                                     start=(mt == 0), stop=(mt == NT - 1))
                rc = rec_pool.tile([128, 1], dt.float32)
                nc.vector.reciprocal(rc, po[:, D:D + 1])
                nc.vector.tensor_scalar_mul(out_p[pp][:, nt, :], po[:, 0:D], rc)

            emit_dmas(0)
            emit_ab_chunk(0, 0)
            for p in range(HPC):
                qk_sb = qk_tiles[p]
                ex_tiles[p] = []
                for t in range(NT):
                    ex = ex_pool.tile([128, N], dt.bfloat16, tag="ex")
                    ex_tiles[p].append(ex)
                    for h2 in range(2):
                        eT = psum_e.tile([128, 1024], dt.float32, tag="eT")
                        for cc in range(2):
                            nc.tensor.matmul(
                                eT[:, cc * 512:(cc + 1) * 512],
                                ab_tiles[p][t // 4][:, (t % 4) * 128:(t % 4 + 1) * 128],
                                qk_sb[:, h2 * 1024 + cc * 512: h2 * 1024 + (cc + 1) * 512],
                                start=True, stop=True)
                        nc.scalar.activation(ex[:, h2 * 1024:(h2 + 1) * 1024],
                                             eT, AF.Exp, scale=0.125)
                    if p > 0:
                        emit_v(p - 1, t)
                        if t == NT - 1:
                            nc.sync.dma_start(out=out_d[p - 1], in_=out_p[p - 1])
                    if p == 0 and t in (2, 6, 10):
                        # after this t's eT matmuls so the prep doesn't delay
                        # the exp pipeline (PE program order)
                        emit_ab_chunk(0, (t + 2) // 4)
                    if 4 <= t < 8:
                        emit_vp_chunk(p, t - 4)
                    if p < HPC - 1:
                        if t == 7:
                            emit_dmas(p + 1)
                        elif 8 <= t < 12:
                            emit_ab_chunk(p + 1, t - 8)
                if p > 0:
                    del ex_tiles[p - 1]

            # tail drain, pair 3: group 4 n-tiles per PSUM tile so PE runs a
            # 64-matmul chain while DVE normalizes the previous group.
            pl = HPC - 1
            for g0, gn in ((0, 4), (4, 4), (8, 4), (12, 4)):
                po4 = psum_o.tile([128, gn, D + 1], dt.float32, tag="po")
                for j in range(gn):
                    nt = g0 + j
                    for mt in range(NT):
                        nc.tensor.matmul(
                            po4[:, j, :], ex_tiles[pl][mt][:, nt * 128:(nt + 1) * 128],
                            vp_tiles[pl][mt // 4][:, mt % 4, :],
                            start=(mt == 0), stop=(mt == NT - 1))
                rc4 = rec_pool.tile([128, gn], dt.float32)
                nc.vector.reciprocal(rc4, po4[:, :, D])
                for j in range(gn):
                    # ACT is idle after the last exp: split the normalize muls
                    # between ACT (out = Copy(in * scale)) and DVE
                    if j % 2 == 0:
                        nc.scalar.activation(out_p[pl][:, g0 + j, :],
                                             po4[:, j, 0:D], AF.Copy,
                                             scale=rc4[:, j:j + 1])
                    else:
                        nc.vector.tensor_scalar_mul(
                            out_p[pl][:, g0 + j, :], po4[:, j, 0:D],
                            rc4[:, j:j + 1])
                nc.sync.dma_start(out=out_d[pl, :, g0:g0 + gn, :],
                                  in_=out_p[pl][:, g0:g0 + gn, :])
    nc.finalize()
    return nc


_NC_CACHE = None


def kernel(q, k, v, xyz, W1, W2, W3):
    global _NC_CACHE
    if _NC_CACHE is None:
        _NC_CACHE = _build_nc()
    nc = _NC_CACHE

    in_maps = []
    for c in range(NCORES):
        b, h0 = c // 2, (c % 2) * HPC
        qh = q[b, h0:h0 + HPC]                       # [4, 2048, 64] f32
        kh = k[b, h0:h0 + HPC]
        vh = v[b, h0:h0 + HPC]
        qk = np.concatenate(
            [qh.transpose(0, 2, 1), kh.transpose(0, 2, 1)], axis=1
        ).astype(_bf16)                              # [4, 128, 2048]
        vt = np.ascontiguousarray(
            vh.reshape(HPC, NT, 128, D).transpose(0, 2, 1, 3)
        ).reshape(HPC, 128, NT * D).astype(_bf16)    # [4, 128, 1024]
        ws = slice(h0 * D, (h0 + HPC) * D)
        wx = np.concatenate(
            [xyz[b], W1[ws].T, W2[ws].T, W3[ws].T], axis=1).astype(_bf16)
        in_maps.append({
            "qk": np.ascontiguousarray(qk),
            "vt": np.ascontiguousarray(vt),
            "wx": np.ascontiguousarray(wx),
        })

    res = run_bass_kernel_spmd(nc, in_maps, list(range(NCORES)))
    outs = res.results

    full = np.empty((B, N, H, D), dtype=np.float32)
    for c in range(NCORES):
        b, h0 = c // 2, (c % 2) * HPC
        oc = np.asarray(outs[c]["out"])              # [4, 128, 16, 64]
        full[b, :, h0:h0 + HPC, :] = (
            oc.transpose(2, 1, 0, 3).reshape(N, HPC, D)
        )
    return full


# revision 44
# speedup vs baseline: 1.0130x; 1.0130x over previous
import numpy as np
import ml_dtypes

from concourse import bass, tile
from concourse import bacc
from concourse import mybir
from concourse.bass_utils import run_bass_kernel_spmd
from concourse.masks import make_identity

dt = mybir.dt
AF = mybir.ActivationFunctionType

B, H, N, D = 4, 8, 2048, 64
NCORES = 8
HPC = 4          # heads per core
NT = N // 128    # 16 n-tiles of 128

_bf16 = ml_dtypes.bfloat16


def _build_nc():
    nc = bacc.Bacc("TRN2", target_bir_lowering=False)
    qk_d = nc.dram_tensor("qk", [HPC, 128, N], dt.bfloat16, kind="ExternalInput")
    v_d = nc.dram_tensor("vt", [HPC, 128, NT * D], dt.bfloat16, kind="ExternalInput")
    # xyz | W1.T | W2.T | W3.T packed: one DMA descriptor instead of four
    # (HWDGE descriptor-gen is a shared 625ns/DMA resource on the startup
    # critical path).
    wx_d = nc.dram_tensor("wx", [3, N + 3 * HPC * D], dt.bfloat16,
                          kind="ExternalInput")
    # kT rows of pair 0, cols 0:512, duplicated at partition base 0 so the
    # startup eT tiles can take kT as lhsT with qT (base 0) as rhs.
    kd_d = nc.dram_tensor("kd", [64, 512], dt.bfloat16, kind="ExternalInput")
    out_d = nc.dram_tensor("out", [HPC, 128, NT, D], dt.float32,
                           kind="ExternalOutput")
    W1O, W2O, W3O = N, N + HPC * D, N + 2 * HPC * D

    with tile.TileContext(nc) as tc:
        with (
            tc.tile_pool(name="const", bufs=1) as cpool,
            tc.tile_pool(name="qk", bufs=2) as qk_pool,
            tc.tile_pool(name="ab", bufs=8) as ab_pool,
            tc.tile_pool(name="vraw", bufs=2) as vraw_pool,
            tc.tile_pool(name="vp", bufs=8) as vp_pool,
            tc.tile_pool(name="expb", bufs=2 * NT) as ex_pool,
            tc.tile_pool(name="rec", bufs=4) as rec_pool,
            tc.tile_pool(name="pse", bufs=2, space="PSUM") as psum_e,
            tc.tile_pool(name="psp", bufs=2, space="PSUM") as psum_p,
            tc.tile_pool(name="pso", bufs=2, space="PSUM") as psum_o,
        ):
            # PE warmup: pe_busy_start is latched at the first matmul and the
            # clock ramps to peak 3us later regardless of gaps, so two tiny
            # early matmuls start the ramp clock ASAP.
            z128 = cpool.tile([128, 128], dt.bfloat16)
            nc.vector.memset(z128, 0.0)
            warm = psum_p.tile([128, 128], dt.float32, tag="pp")
            for i in range(2):
                nc.tensor.matmul(warm, z128, z128, start=True, stop=True)

            ident_bf = cpool.tile([128, 128], dt.bfloat16)
            make_identity(nc, ident_bf)
            wx_sb = cpool.tile([3, N + 3 * HPC * D], dt.bfloat16)
            # first on the sync HWDGE queue: gates the first ab chunk
            nc.sync.dma_start(out=wx_sb, in_=wx_d[:])
            xyz_sb = wx_sb[:, 0:N]
            out_p = [cpool.tile([128, NT, D], dt.float32, name=f"out_p{i}")
                     for i in range(HPC)]

            ex_tiles: dict[int, list] = {}
            vp_tiles: dict[int, list] = {}
            ab_tiles: dict[int, list] = {}
            qk_tiles: dict[int, object] = {}
            v_tiles: dict[int, object] = {}

            def emit_dmas(p, kd_sb=None):
                # qk in two halves so the first eT matmuls can start after
                # half the transfer (matters for pair 0 on the startup path)
                qk_sb = qk_pool.tile([128, N], dt.bfloat16)
                nc.sync.dma_start(out=qk_sb[:, 0:N // 2], in_=qk_d[p, :, 0:N // 2])
                if kd_sb is not None:
                    nc.sync.dma_start(out=kd_sb, in_=kd_d[:])
                nc.sync.dma_start(out=qk_sb[:, N // 2:N], in_=qk_d[p, :, N // 2:N])
                qk_tiles[p] = qk_sb
                v_sb = vraw_pool.tile([128, NT, D], dt.bfloat16)
                nc.sync.dma_start(out=v_sb, in_=v_d[p])
                v_tiles[p] = v_sb
                ab_tiles[p] = []
                vp_tiles[p] = []

            def emit_ab_chunk(p, c4):
                # AB = vstack(kT + q_pe, k_pe), bf16.  kT folded in via PE
                # identity-accumulate so the DVE copy has a single producer.
                hs = slice(p * D, (p + 1) * D)
                s = slice(c4 * 512, (c4 + 1) * 512)
                qk_sb = qk_tiles[p]
                pp = psum_p.tile([128, 512], dt.float32, tag="pp")
                nc.tensor.matmul(pp[0:64], wx_sb[:, W1O + p * D:W1O + (p + 1) * D],
                                 xyz_sb[:, s], start=True, stop=False)
                nc.tensor.matmul(pp[0:64], ident_bf[64:128, 64:128],
                                 qk_sb[64:128, s], start=False, stop=True)
                nc.tensor.matmul(pp[64:128], wx_sb[:, W3O + p * D:W3O + (p + 1) * D],
                                 xyz_sb[:, s], start=True, stop=True)
                ab = ab_pool.tile([128, 512], dt.bfloat16, tag="ab")
                nc.vector.tensor_copy(ab, pp)
                ab_tiles[p].append(ab)

            def emit_vp_chunk(p, c4):
                # vp = [v + v_peT | 1], bf16  [128, 4, 65]; v folded in via PE
                pv = psum_p.tile([128, 4, D], dt.float32, tag="pp")
                for j in range(4):
                    t = c4 * 4 + j
                    nc.tensor.matmul(pv[:, j, :], xyz_sb[:, t * 128:(t + 1) * 128],
                                     wx_sb[:, W2O + p * D:W2O + (p + 1) * D],
                                     start=True, stop=False)
                    nc.tensor.matmul(pv[:, j, :], ident_bf,
                                     v_tiles[p][:, t, :], start=False, stop=True)
                vp = vp_pool.tile([128, 4, D + 1], dt.bfloat16, tag="vp")
                nc.vector.memset(vp[:, :, D], 1.0)
                nc.vector.tensor_copy(vp[:, :, 0:D], pv)
                vp_tiles[p].append(vp)

            def emit_v(pp, nt):
                # direct-layout V pass: po[n, d] = sum_m ex[m, n] * vp[m, d];
                # col 64 (ones column of vp) = softmax denominator per row.
                po = psum_o.tile([128, D + 1], dt.float32, tag="po")
                for mt in range(NT):
                    nc.tensor.matmul(po, ex_tiles[pp][mt][:, nt * 128:(nt + 1) * 128],
                                     vp_tiles[pp][mt // 4][:, mt % 4, :],
                                     start=(mt == 0), stop=(mt == NT - 1))
                rc = rec_pool.tile([128, 1], dt.float32)
                nc.vector.reciprocal(rc, po[:, D:D + 1])
                nc.vector.tensor_scalar_mul(out_p[pp][:, nt, :], po[:, 0:D], rc)

            kd_sb = cpool.tile([64, 512], dt.bfloat16)
            emit_dmas(0, kd_sb=kd_sb)
            # pair-0 chunk 0: wx-only pe terms (no kT fold, no qk dependency);
            # the kT@qT term is added directly in the eT accumulation below so
            # the startup chain is gated only by the small wx DMA.
            ppe = psum_p.tile([128, 512], dt.float32, tag="pp")
            nc.tensor.matmul(ppe[0:64], wx_sb[:, W1O:W1O + D],
                             xyz_sb[:, 0:512], start=True, stop=True)
            nc.tensor.matmul(ppe[64:128], wx_sb[:, W3O:W3O + D],
                             xyz_sb[:, 0:512], start=True, stop=True)
            abpe0 = ab_pool.tile([128, 512], dt.bfloat16, tag="ab")
            nc.scalar.activation(abpe0, ppe, AF.Copy)
            ab_tiles[0].append(None)
            for p in range(HPC):
                qk_sb = qk_tiles[p]
                ex_tiles[p] = []
                for t in range(NT):
                    ex = ex_pool.tile([128, N], dt.bfloat16, tag="ex")
                    ex_tiles[p].append(ex)
                    for h2 in range(2):
                        eT = psum_e.tile([128, 1024], dt.float32, tag="eT")
                        for cc in range(2):
                            cs = slice(h2 * 1024 + cc * 512,
                                       h2 * 1024 + (cc + 1) * 512)
                            if p == 0 and t < 4:
                                nc.tensor.matmul(
                                    eT[:, cc * 512:(cc + 1) * 512],
                                    abpe0[:, t * 128:(t + 1) * 128],
                                    qk_sb[:, cs], start=True, stop=False)
                                nc.tensor.matmul(
                                    eT[:, cc * 512:(cc + 1) * 512],
                                    kd_sb[:, t * 128:(t + 1) * 128],
                                    qk_sb[0:64, cs], start=False, stop=True)
                            else:
                                nc.tensor.matmul(
                                    eT[:, cc * 512:(cc + 1) * 512],
                                    ab_tiles[p][t // 4][:, (t % 4) * 128:(t % 4 + 1) * 128],
                                    qk_sb[:, cs],
                                    start=True, stop=True)
                        nc.scalar.activation(ex[:, h2 * 1024:(h2 + 1) * 1024],
                                             eT, AF.Exp, scale=0.125)
                    if p > 0:
                        emit_v(p - 1, t)
                        if t == NT - 1:
                            nc.sync.dma_start(out=out_d[p - 1], in_=out_p[p - 1])
                    if p == 0 and t in (2, 6, 10):
                        # gated past startup so the scheduler can't place the
                        # chunk matmuls ahead of the ready eT chain
                        with tc.tile_wait_until(0.008):
                            emit_ab_chunk(0, (t + 2) // 4)
                    if 4 <= t < 8:
                        # keep pair-0 vp prep (v-DMA-gated) from being
                        # statically scheduled ahead of the ready eT matmuls
                        # on the startup critical path
                        with tc.tile_wait_until(0.01, enable=(p == 0)):
                            emit_vp_chunk(p, t - 4)
                    if p < HPC - 1:
                        if t == 7:
                            emit_dmas(p + 1)
                        elif 8 <= t < 12:
                            emit_ab_chunk(p + 1, t - 8)
                if p > 0:
                    del ex_tiles[p - 1]

            # tail drain, pair 3: group 4 n-tiles per PSUM tile so PE runs a
            # 64-matmul chain while DVE normalizes the previous group.
            pl = HPC - 1
            for g0, gn in ((0, 4), (4, 4), (8, 4), (12, 4)):
                po4 = psum_o.tile([128, gn, D + 1], dt.float32, tag="po")
                for j in range(gn):
                    nt = g0 + j
                    for mt in range(NT):
                        nc.tensor.matmul(
                            po4[:, j, :], ex_tiles[pl][mt][:, nt * 128:(nt + 1) * 128],
                            vp_tiles[pl][mt // 4][:, mt % 4, :],
                            start=(mt == 0), stop=(mt == NT - 1))
                rc4 = rec_pool.tile([128, gn], dt.float32)
                nc.vector.reciprocal(rc4, po4[:, :, D])
                for j in range(gn):
                    # ACT is idle after the last exp: split the normalize muls
                    # between ACT (out = Copy(in * scale)) and DVE
                    if j % 2 == 0:
                        nc.scalar.activation(out_p[pl][:, g0 + j, :],
                                             po4[:, j, 0:D], AF.Copy,
                                             scale=rc4[:, j:j + 1])
                    else:
                        nc.vector.tensor_scalar_mul(
                            out_p[pl][:, g0 + j, :], po4[:, j, 0:D],
                            rc4[:, j:j + 1])
                nc.sync.dma_start(out=out_d[pl, :, g0:g0 + gn, :],
                                  in_=out_p[pl][:, g0:g0 + gn, :])
    nc.finalize()
    return nc


_NC_CACHE = None


def kernel(q, k, v, xyz, W1, W2, W3):
    global _NC_CACHE
    if _NC_CACHE is None:
        _NC_CACHE = _build_nc()
    nc = _NC_CACHE

    in_maps = []
    for c in range(NCORES):
        b, h0 = c // 2, (c % 2) * HPC
        qh = q[b, h0:h0 + HPC]                       # [4, 2048, 64] f32
        kh = k[b, h0:h0 + HPC]
        vh = v[b, h0:h0 + HPC]
        qk = np.concatenate(
            [qh.transpose(0, 2, 1), kh.transpose(0, 2, 1)], axis=1
        ).astype(_bf16)                              # [4, 128, 2048]
        vt = np.ascontiguousarray(
            vh.reshape(HPC, NT, 128, D).transpose(0, 2, 1, 3)
        ).reshape(HPC, 128, NT * D).astype(_bf16)    # [4, 128, 1024]
        ws = slice(h0 * D, (h0 + HPC) * D)
        wx = np.concatenate(
            [xyz[b], W1[ws].T, W2[ws].T, W3[ws].T], axis=1).astype(_bf16)
        in_maps.append({
            "qk": np.ascontiguousarray(qk),
            "vt": np.ascontiguousarray(vt),
            "wx": np.ascontiguousarray(wx),
            "kd": np.ascontiguousarray(qk[0, 64:128, 0:512]),
        })

    res = run_bass_kernel_spmd(nc, in_maps, list(range(NCORES)))
    outs = res.results

    full = np.empty((B, N, H, D), dtype=np.float32)
    for c in range(NCORES):
        b, h0 = c // 2, (c % 2) * HPC
        oc = np.asarray(outs[c]["out"])              # [4, 128, 16, 64]
        full[b, :, h0:h0 + HPC, :] = (
            oc.transpose(2, 1, 0, 3).reshape(N, HPC, D)
        )
    return full
